# revision 11
# baseline (speedup 1.0000x reference)
import sys
import time

sys.path.insert(0, "/opt/trn_rl_repo")

import numpy as np

from concourse import bass, bacc, tile, mybir
from concourse.bass_utils import run_bass_kernel_spmd

# Problem constants (nn_ClassicalLSTMCell): hardcoded per harness contract.
T, B, D_IN, D_H = 1024, 128, 256, 256
N_CORES = 8
B_LOC = B // N_CORES          # 16 sequences per core
G = 4 * D_H                   # 1024 gate dims, col order [f, i, o, g]
KX = D_IN // 128              # 2 k-chunks for x
KH = D_H // 128               # 2 k-chunks for h
M = G // 128                  # 8 m-chunks of gate dims
SPT = 16                      # steps per t-tile
BODY_TT = 4                   # t-tiles per For_i iteration (64 steps)

FP32 = mybir.dt.float32
AF = mybir.ActivationFunctionType

PRE_W = SPT * B_LOC * M       # 2048 pre cols per t-tile
HST_W = SPT * KH * B_LOC      # 512 hout cols per t-tile
CW = KH * B_LOC               # 32 state cols


def build(nc, t_steps=T, static_body=False, reps=1):
    """Emit the LSTM kernel IR for one core into `nc`."""
    ntt = t_steps // SPT
    assert ntt % BODY_TT == 0
    n_iter = ntt // BODY_TT
    xT = nc.dram_tensor("xT", [D_IN, t_steps * B_LOC], FP32, kind="ExternalInput").ap()
    wx = nc.dram_tensor("wx", [D_IN, G], FP32, kind="ExternalInput").ap()
    wh = nc.dram_tensor("wh", [D_H, G], FP32, kind="ExternalInput").ap()
    bg = nc.dram_tensor("bg", [128, M], FP32, kind="ExternalInput").ap()
    hout = nc.dram_tensor("hout", [ntt, 128, HST_W], FP32, kind="ExternalOutput").ap()
    cout = nc.dram_tensor("cout", [128, CW], FP32, kind="ExternalOutput").ap()

    from contextlib import ExitStack, nullcontext

    with tile.TileContext(nc) as tc:
        with tc.tile_pool(name="dram", bufs=1, space="DRAM") as dpool:
            # pre[tt, p, m*256 + s*16 + b]; padded for harmless OOB prefetch
            pre_d = dpool.tile([ntt + BODY_TT, 128, PRE_W], FP32)

            rep_cm = tc.For_i(0, reps) if reps > 1 else nullcontext()
            rep_cm.__enter__()

            # ---------------- Phase A: pre = x @ Wx + b ----------------
            NT = SPT * B_LOC  # 256 cols per n-tile == one t-tile
            with (
                tc.tile_pool(name="awt", bufs=1) as awt,
                tc.tile_pool(name="ax", bufs=3) as ax,
                tc.tile_pool(name="aout", bufs=3) as aout,
                tc.tile_pool(name="apsum", bufs=4, space="PSUM") as apsum,
            ):
                wx_s = awt.tile([128, KX * G], FP32)
                for k in range(KX):
                    nc.sync.dma_start(
                        wx_s[:, k * G : (k + 1) * G], wx[k * 128 : (k + 1) * 128, :]
                    )
                b_s = awt.tile([128, M], FP32)
                nc.sync.dma_start(b_s[:], bg[:])

                for nt in range(ntt):
                    xs = ax.tile([128, KX * NT], FP32)
                    for k in range(KX):
                        nc.sync.dma_start(
                            xs[:, k * NT : (k + 1) * NT],
                            xT[k * 128 : (k + 1) * 128, nt * NT : (nt + 1) * NT],
                        )
                    ot = aout.tile([128, M * NT], FP32)
                    for m in range(M):
                        ps = apsum.tile([128, NT], FP32)
                        for k in range(KX):
                            nc.tensor.matmul(
                                ps[:],
                                wx_s[:, k * G + m * 128 : k * G + (m + 1) * 128],
                                xs[:, k * NT : (k + 1) * NT],
                                start=(k == 0),
                                stop=(k == KX - 1),
                            )
                        nc.vector.tensor_scalar_add(
                            ot[:, m * NT : (m + 1) * NT], ps[:], b_s[:, m : m + 1]
                        )
                    nc.sync.dma_start(pre_d[nt], ot[:])

            # ---------------- Phase B: recurrence ----------------
            with (
                tc.tile_pool(name="bwt", bufs=1) as bwt,
                tc.tile_pool(name="bz", bufs=3) as bz,
                tc.tile_pool(name="bsm", bufs=3) as bsm,
                tc.tile_pool(name="bpsum", bufs=4, space="PSUM") as bpsum,
            ):
                wh_s = bwt.tile([128, KH * G], FP32)
                for k in range(KH):
                    nc.sync.dma_start(
                        wh_s[:, k * G : (k + 1) * G], wh[k * 128 : (k + 1) * 128, :]
                    )
                H2 = BODY_TT // 2
                preA = bwt.tile([128, H2 * PRE_W], FP32)
                preB = bwt.tile([128, H2 * PRE_W], FP32)
                hstA = bwt.tile([128, H2 * HST_W], FP32)
                hstB = bwt.tile([128, H2 * HST_W], FP32)
                cE = bwt.tile([128, CW], FP32)
                cO = bwt.tile([128, CW], FP32)
                nc.vector.memset(cO[:], 0.0)
                nc.vector.memset(hstB[:], 0.0)

                # prologue: load first half of iteration 0
                for j in range(H2):
                    nc.sync.dma_start(
                        preA[:, j * PRE_W : (j + 1) * PRE_W], pre_d[j]
                    )

                def steps(pre_t, hst, g0):
                    """One half-body: H2 t-tiles = H2*SPT steps."""
                    for ls in range(H2 * SPT):
                        g = g0 + ls
                        # h location of previous step (hstB last slice is
                        # zero-initialized for the very first step)
                        if ls == 0:
                            hp, off = (hstA if g0 == H2 * SPT else hstB), (
                                H2 * SPT - 1
                            ) * CW
                        else:
                            hp, off = hst, (ls - 1) * CW
                        c_rd = cO if g % 2 == 0 else cE
                        c_wr = cE if g % 2 == 0 else cO

                        ps = bpsum.tile([128, M * B_LOC], FP32)
                        for m in range(M):
                            for k in range(KH):
                                nc.tensor.matmul(
                                    ps[:, m * B_LOC : (m + 1) * B_LOC],
                                    wh_s[:, k * G + m * 128 : k * G + (m + 1) * 128],
                                    hp[:, off + k * B_LOC : off + (k + 1) * B_LOC],
                                    start=(k == 0),
                                    stop=(k == KH - 1),
                                )
                        zs = bz.tile([128, M * B_LOC], FP32)
                        # pre tile covers H2 t-tiles: free = j*PRE_W + m*256 + s*16 + b
                        j, s = divmod(ls, SPT)
                        pv = pre_t[:, j * PRE_W : (j + 1) * PRE_W].rearrange(
                            "p (m sb) -> p m sb", m=M
                        )[:, :, s * B_LOC : (s + 1) * B_LOC]
                        nc.vector.tensor_add(
                            zs[:].rearrange("p (m b) -> p m b", m=M),
                            ps[:].rearrange("p (m b) -> p m b", m=M),
                            pv,
                        )
                        sig = bsm.tile([128, 6 * B_LOC], FP32, tag="sig")
                        nc.scalar.activation(sig[:], zs[:, : 6 * B_LOC], AF.Sigmoid)
                        gt = bsm.tile([128, CW], FP32, tag="gt")
                        nc.scalar.activation(gt[:], zs[:, 6 * B_LOC :], AF.Tanh)
                        t1 = bsm.tile([128, CW], FP32, tag="t1")
                        nc.vector.tensor_mul(t1[:], sig[:, 2 * B_LOC : 4 * B_LOC], gt[:])
                        t2 = bsm.tile([128, CW], FP32, tag="t2")
                        nc.vector.tensor_mul(t2[:], sig[:, : 2 * B_LOC], c_rd[:])
                        nc.vector.tensor_add(c_wr[:], t1[:], t2[:])
                        tch = bsm.tile([128, CW], FP32, tag="tch")
                        nc.scalar.activation(tch[:], c_wr[:], AF.Tanh)
                        nc.vector.tensor_mul(
                            hst[:, ls * CW : (ls + 1) * CW],
                            sig[:, 4 * B_LOC : 6 * B_LOC],
                            tch[:],
                        )

                def emit_body(base):
                    """base: ScalarValue expr or int (static)."""
                    def dix(off):
                        return bass.ds(base + off, 1) if not isinstance(base, int) else bass.ds(base + off, 1)

                    for j in range(H2):
                        nc.sync.dma_start(
                            preB[:, j * PRE_W : (j + 1) * PRE_W],
                            pre_d[dix(H2 + j), :, :],
                        )
                    steps(preA, hstA, 0)
                    for j in range(H2):
                        nc.sync.dma_start(
                            hout[dix(j), :, :],
                            hstA[:, j * HST_W : (j + 1) * HST_W],
                        )
                        nc.sync.dma_start(
                            preA[:, j * PRE_W : (j + 1) * PRE_W],
                            pre_d[dix(BODY_TT + j), :, :],
                        )
                    steps(preB, hstB, H2 * SPT)
                    for j in range(H2):
                        nc.sync.dma_start(
                            hout[dix(H2 + j), :, :],
                            hstB[:, j * HST_W : (j + 1) * HST_W],
                        )

                if static_body:
                    for i in range(n_iter):
                        emit_body(i * BODY_TT)
                else:
                    with tc.For_i(
                        0,
                        n_iter,
                        hint_engines=(mybir.EngineType.PE, mybir.EngineType.DVE),
                    ) as it:
                        emit_body(it * BODY_TT)
                nc.sync.dma_start(cout[:], cO[:])
            rep_cm.__exit__(None, None, None)
    nc.compile()


def prep_inputs(inputs, Wf, bf, Wi, bi, Wg, bg_, Wo, bo, t_steps=T):
    """Host-side layout prep. Returns list of per-core input dicts."""
    W = np.concatenate([Wf, Wi, Wo, Wg], axis=1).astype(np.float32)  # [512, 1024]
    bias = np.concatenate([bf, bi, bo, bg_]).astype(np.float32)      # [1024]
    wx = np.ascontiguousarray(W[:D_IN])
    wh = np.ascontiguousarray(W[D_IN:])
    bmat = np.ascontiguousarray(bias.reshape(M, 128).T)
    maps = []
    for c in range(N_CORES):
        sl = inputs[:t_steps, c * B_LOC : (c + 1) * B_LOC, :]
        xT = np.ascontiguousarray(
            sl.transpose(2, 0, 1).reshape(D_IN, t_steps * B_LOC)
        ).astype(np.float32)
        maps.append({"xT": xT, "wx": wx, "wh": wh, "bg": bmat})
    return maps


def decode_outputs(results, t_steps=T):
    ntt = t_steps // SPT
    outs = np.empty((t_steps, B, D_H), np.float32)
    cx = np.empty((B, D_H), np.float32)
    for c in range(N_CORES):
        ho = results[c]["hout"].reshape(ntt, 128, SPT, KH, B_LOC)
        outs[:, c * B_LOC : (c + 1) * B_LOC, :] = (
            ho.transpose(0, 2, 4, 3, 1).reshape(t_steps, B_LOC, D_H)
        )
        co = results[c]["cout"].reshape(128, KH, B_LOC)
        cx[c * B_LOC : (c + 1) * B_LOC, :] = co.transpose(2, 1, 0).reshape(B_LOC, D_H)
    hx = outs[-1].copy()
    return outs, (hx, cx)


_CACHE = {}


def _get_nc(t_steps=T, reps=1):
    key = (t_steps, reps)
    if key not in _CACHE:
        nc = bacc.Bacc(
            "TRN2", target_bir_lowering=False, debug=False, num_devices=N_CORES
        )
        build(nc, t_steps, reps=reps)
        _CACHE[key] = nc
    return _CACHE[key]


def run(inputs, Wf, bf, Wi, bi, Wg, bg, Wo, bo, t_steps=T, trace=False, reps=1):
    nc = _get_nc(t_steps, reps)
    maps = prep_inputs(inputs, Wf, bf, Wi, bi, Wg, bg, Wo, bo, t_steps)
    res = run_bass_kernel_spmd(nc, maps, list(range(N_CORES)), trace=trace)
    return decode_outputs(res.results, t_steps), res


def kernel(inputs, Wf, bf, Wi, bi, Wg, bg, Wo, bo):
    (outs, (hx, cx)), _ = run(inputs, Wf, bf, Wi, bi, Wg, bg, Wo, bo)
    return outs, (hx, cx)


if __name__ == "__main__":
    t_steps = int(sys.argv[1]) if len(sys.argv) > 1 else 64
    t0 = time.time()
    _get_nc(t_steps)
    print(f"build+compile T={t_steps}: {time.time()-t0:.1f}s")
